# revision 6
# baseline (speedup 1.0000x reference)
"""MoE dense all-experts (GPT-OSS Experts forward) on 8 Trainium2 NeuronCores.

Expert-parallel sharding: core e holds expert e's weights and computes its
weighted contribution

    partial_e[t, h] = w[t, e] * ((up + 1) * silu(1.702 * gate) @ down_e.T + db_e)

with [gate | up] = hs @ gup_e + bias (the host de-interleaves gup's even/odd
columns so gate/up become contiguous halves). Each core writes its full
[T, H] partial to DRAM as it is produced and the host sums the 8 partials in
fp32 — there are no on-device collectives, so the cores run fully decoupled
and the kernel ends right after the last tile's store.

Matmuls run in bf16 (both operands, fp32 PSUM accumulation). The fp32r
version drew enough power that the PE was firmware-throttled to K=13/16
(~1.95 GHz) for the whole kernel; bf16 halves the multiplier energy and the
SBUF/DMA traffic, letting the PE hold its full 2.4 GHz clock. End-to-end
relative error is ~3.6e-3 (gate is 2e-2).

All device tensors are host-pre-transposed into the exact SBUF layouts
(partition-major, fully contiguous) so every DMA moves multi-KB runs per
partition — a strided gather here is descriptor-bound and ~4x slower. The
gate weights are laid out j-major (all K-slices of one 128-column output
strip adjacent) and streamed strip-by-strip so the first matmul chain waits
only for its own strip, not the whole gate half; hidden states stream
chunk-major on the Scalar engine's DMA queue so their issue overlaps the
weight issues on the Sync queue. Stage 1 computes [f, t] tiles (gate pass
feeding the ScalarE Silu LUT, then up pass fused with the silu output via
scalar_tensor_tensor into bf16 act[i, t]); stage 2 computes out[t, h] with
act as the stationary operand. The down-bias + routing-weight epilogue runs
on the VectorE: out = (psum * w[t]) + w[t]*db[h], with the rank-1 w*db tile
built from a partition-broadcast copy of db.
"""
import sys
if '/opt/trn_rl_repo' not in sys.path:
    sys.path.insert(0, '/opt/trn_rl_repo')
import numpy as np
import ml_dtypes

E, H, I, T = 8, 1024, 1024, 4096
N_CORES = 8
NCH = 8
TC = 512               # tokens per chunk (T == NCH * TC)
KC = H // 128          # contraction chunks (H == I == 1024)
NJ = I // 128          # gate/up row tiles
NTT = TC // 128

_CACHE = {}


def _build():
    import concourse.bacc as bacc
    import concourse.tile as tile
    import concourse.mybir as mybir
    f32 = mybir.dt.float32
    bf16 = mybir.dt.bfloat16
    AF = mybir.ActivationFunctionType
    ALU = mybir.AluOpType

    nc = bacc.Bacc("TRN2", target_bir_lowering=False, debug=False,
                   enable_asserts=False, num_devices=N_CORES)
    # all pre-transposed to SBUF layout on the host (see _make_in_maps)
    hsf = nc.dram_tensor("hsf", [128, NCH * KC * TC], bf16, kind="ExternalInput").ap()
    gupg = nc.dram_tensor("gupg", [128, NJ * KC * 128], bf16, kind="ExternalInput").ap()
    gupu = nc.dram_tensor("gupu", [128, KC * I], bf16, kind="ExternalInput").ap()
    dwTf = nc.dram_tensor("dwTf", [128, KC * H], bf16, kind="ExternalInput").ap()
    gb = nc.dram_tensor("gb", [128, NJ], f32, kind="ExternalInput").ap()
    ub = nc.dram_tensor("ub", [128, NJ], f32, kind="ExternalInput").ap()
    db = nc.dram_tensor("db", [1, H], f32, kind="ExternalInput").ap()
    wt = nc.dram_tensor("wt", [128, T // 128], f32, kind="ExternalInput").ap()
    opart = nc.dram_tensor("opart", [T, H], f32, kind="ExternalOutput").ap()

    SJ = KC * 128        # columns per j-strip of the gate half

    with tile.TileContext(nc) as tc_:
        with tc_.tile_pool(name="wpool", bufs=1) as wpool, \
             tc_.tile_pool(name="hpool", bufs=2) as hpool, \
             tc_.tile_pool(name="apool", bufs=2) as apool, \
             tc_.tile_pool(name="spool", bufs=8) as spool, \
             tc_.tile_pool(name="opool", bufs=3) as opool, \
             tc_.tile_pool(name="bpool", bufs=4) as bpool, \
             tc_.tile_pool(name="ps1", bufs=2, space="PSUM") as ps1, \
             tc_.tile_pool(name="ps2", bufs=3, space="PSUM") as ps2:

            gupg_r = wpool.tile([128, NJ * SJ], bf16)
            gupu_r = wpool.tile([128, KC * I], bf16)
            dwT_r = wpool.tile([128, KC * H], bf16)
            gb_r = wpool.tile([128, NJ], f32)
            ub_r = wpool.tile([128, NJ], f32)
            db_f = wpool.tile([1, H], f32)
            db_bc = wpool.tile([128, H], f32)
            w_r = wpool.tile([128, T // 128], f32)

            # Weight stream on the Sync queue, token stream on the Scalar
            # queue — the ~0.6us per-DMA issue costs overlap. Order matches
            # consumption: j=0 and j=1 gate strips first (the opening matmul
            # chains), biases for the first silu, the remaining gate strips
            # paired, then up / routing / down tensors in use order.
            hs0 = hpool.tile([128, KC * TC], bf16, tag="hs")
            nc.scalar.dma_start(hs0[:, 0:TC], hsf[:, 0:TC])
            nc.scalar.dma_start(hs0[:, TC:KC * TC], hsf[:, TC:KC * TC])
            hs1 = hpool.tile([128, KC * TC], bf16, tag="hs")
            nc.scalar.dma_start(hs1[:], hsf[:, KC * TC:2 * KC * TC])

            nc.sync.dma_start(gupg_r[:, 0:SJ], gupg[:, 0:SJ])
            nc.sync.dma_start(gupg_r[:, SJ:2 * SJ], gupg[:, SJ:2 * SJ])
            nc.sync.dma_start(gb_r[:], gb[:])
            nc.sync.dma_start(ub_r[:], ub[:])
            for j2 in range(1, NJ // 2):
                nc.sync.dma_start(gupg_r[:, 2*j2*SJ:2*(j2+1)*SJ],
                                  gupg[:, 2*j2*SJ:2*(j2+1)*SJ])
            nc.sync.dma_start(gupu_r[:], gupu[:])
            nc.sync.dma_start(db_f[:], db[:])
            nc.sync.dma_start(w_r[:], wt[:])
            nc.sync.dma_start(dwT_r[:], dwTf[:])
            nc.gpsimd.partition_broadcast(db_bc[:], db_f[:])

            for c in range(NCH):
                if c == 0:
                    hs_r = hs0
                elif c == 1:
                    hs_r = hs1
                else:
                    hs_r = hpool.tile([128, KC * TC], bf16, tag="hs")
                    nc.scalar.dma_start(hs_r[:], hsf[:, c*KC*TC:(c+1)*KC*TC])

                act_r = apool.tile([128, NJ * TC], bf16, tag="act")
                s2s = []
                for j in range(NJ):     # gate pass
                    pg = ps1.tile([128, TC], f32, tag="pg")
                    for kc in range(KC):
                        nc.tensor.matmul(pg[:], gupg_r[:, j*SJ + kc*128 : j*SJ + (kc+1)*128],
                                         hs_r[:, kc*TC:(kc+1)*TC],
                                         start=(kc == 0), stop=(kc == KC - 1))
                    s2 = spool.tile([128, TC], f32, tag="s2")
                    nc.scalar.activation(s2[:], pg[:], AF.Silu,
                                         bias=gb_r[:, j:j+1], scale=1.702)
                    s2s.append(s2)
                for j in range(NJ):     # up pass: act = (up + ub + 1) * silu_out
                    pu = ps1.tile([128, TC], f32, tag="pu")
                    for kc in range(KC):
                        nc.tensor.matmul(pu[:], gupu_r[:, kc*I + j*128 : kc*I + (j+1)*128],
                                         hs_r[:, kc*TC:(kc+1)*TC],
                                         start=(kc == 0), stop=(kc == KC - 1))
                    nc.vector.scalar_tensor_tensor(act_r[:, j*TC:(j+1)*TC], pu[:],
                                                   ub_r[:, j:j+1], s2s[j][:],
                                                   op0=ALU.add, op1=ALU.mult)

                for tt in range(NTT):
                    gt = c * NTT + tt
                    wcol = w_r[:, gt:gt+1]
                    ot = opool.tile([128, H], f32, tag="ot")
                    for hh in range(H // 512):
                        dbw = bpool.tile([128, 512], f32, tag="dbw")
                        nc.vector.tensor_scalar_mul(dbw[:], db_bc[:, hh*512:(hh+1)*512], wcol)
                        p2 = ps2.tile([128, 512], f32, tag="p2")
                        for ic in range(KC):
                            nc.tensor.matmul(p2[:], act_r[:, ic*TC + tt*128 : ic*TC + (tt+1)*128],
                                             dwT_r[:, ic*H + hh*512 : ic*H + (hh+1)*512],
                                             start=(ic == 0), stop=(ic == KC - 1))
                        nc.vector.scalar_tensor_tensor(ot[:, hh*512:(hh+1)*512], p2[:], wcol,
                                                       dbw[:], op0=ALU.mult, op1=ALU.add)
                        nc.sync.dma_start(opart[gt*128:(gt+1)*128, hh*512:(hh+1)*512],
                                          ot[:, hh*512:(hh+1)*512])
    nc.compile()
    return nc


def _get_nc():
    if 'nc' not in _CACHE:
        _CACHE['nc'] = _build()
    return _CACHE['nc']


def _make_in_maps(hidden_states, routing_weights, gate_up_proj, gate_up_proj_bias,
                  down_proj, down_proj_bias):
    bf16 = ml_dtypes.bfloat16
    hs = np.asarray(hidden_states, dtype=np.float32)
    rw = np.asarray(routing_weights, dtype=np.float32)
    gupw = np.asarray(gate_up_proj, dtype=np.float32)
    gupb = np.asarray(gate_up_proj_bias, dtype=np.float32)
    dw = np.asarray(down_proj, dtype=np.float32)
    dbias = np.asarray(down_proj_bias, dtype=np.float32)
    # hsT[kc*128+p, c*TC+t] -> hsf[p, c*(KC*TC) + kc*TC + t]  (chunk-major)
    hsT = hs.T.astype(bf16)
    hsf = np.ascontiguousarray(
        hsT.reshape(KC, 128, NCH, TC).transpose(1, 2, 0, 3).reshape(128, NCH * KC * TC))
    in_maps = []
    for e in range(N_CORES):
        g = gupw[e]
        g_gate = (g[:, 0::2]).astype(bf16)   # [H, I]
        g_up = (g[:, 1::2]).astype(bf16)     # [H, I]
        # g_gate[kc*128+p, j*128+c] -> gupg[p, j*(KC*128) + kc*128 + c]  (j-major)
        gupg = np.ascontiguousarray(
            g_gate.reshape(KC, 128, NJ, 128).transpose(1, 2, 0, 3).reshape(128, NJ * KC * 128))
        # g_up[kc*128+p, col] -> gupu[p, kc*I + col]  (kc-major)
        gupu = np.ascontiguousarray(
            g_up.reshape(KC, 128, I).transpose(1, 0, 2).reshape(128, KC * I))
        # dwT[ic*128+p, h] -> dwTf[p, ic*H + h]; silu's 1.702 scale folded in
        dwT = (dw[e].T / np.float32(1.702)).astype(bf16)
        dwTf = np.ascontiguousarray(
            dwT.reshape(KC, 128, H).transpose(1, 0, 2).reshape(128, KC * H))
        in_maps.append({
            "hsf": hsf,
            "gupg": gupg,
            "gupu": gupu,
            "dwTf": dwTf,
            # silu(1.702*(x + b)) = silu(1.702*x + 1.702*b)
            "gb": np.ascontiguousarray((1.702 * gupb[e, 0::2]).reshape(NJ, 128).T),
            "ub": np.ascontiguousarray((gupb[e, 1::2] + 1.0).reshape(NJ, 128).T),
            "db": np.ascontiguousarray(dbias[e][None, :]),
            "wt": np.ascontiguousarray(rw[:, e].reshape(T // 128, 128).T),
        })
    return in_maps


def _assemble(results):
    out = results[0]["opart"].astype(np.float32, copy=True)
    for r in range(1, N_CORES):
        np.add(out, results[r]["opart"], out=out)
    return out


def kernel(hidden_states, routing_weights, gate_up_proj, gate_up_proj_bias,
           down_proj, down_proj_bias):
    from concourse import bass_utils
    in_maps = _make_in_maps(hidden_states, routing_weights, gate_up_proj,
                            gate_up_proj_bias, down_proj, down_proj_bias)
    nc = _get_nc()
    try:
        res = bass_utils.run_bass_kernel_spmd(nc, in_maps, core_ids=list(range(N_CORES)))
    except Exception:
        # One retry in case a previous process left a core wedged.
        res = bass_utils.run_bass_kernel_spmd(nc, in_maps, core_ids=list(range(N_CORES)))
    return _assemble(res.results)


# revision 9
# speedup vs baseline: 1.0012x; 1.0012x over previous
"""MoE dense all-experts (GPT-OSS Experts forward) on 8 Trainium2 NeuronCores.

Expert-parallel sharding: core e holds expert e's weights and computes its
weighted contribution

    partial_e[t, h] = w[t, e] * ((up + 1) * silu(1.702 * gate) @ down_e.T + db_e)

with [gate | up] = hs @ gup_e + bias (the host de-interleaves gup's even/odd
columns so gate/up become contiguous halves). Each core writes its full
[T, H] partial to DRAM as it is produced and the host sums the 8 partials in
fp32 — there are no on-device collectives, so the cores run fully decoupled
and the kernel ends right after the last tile's store.

Matmuls run in bf16 (both operands, fp32 PSUM accumulation). The fp32r
version drew enough power that the PE was firmware-throttled to K=13/16
(~1.95 GHz) for the whole kernel; bf16 halves the multiplier energy and the
SBUF/DMA traffic, letting the PE hold its full 2.4 GHz clock. End-to-end
relative error is ~3.6e-3 (gate is 2e-2).

All device tensors are host-pre-transposed into the exact SBUF layouts
(partition-major, fully contiguous) so every DMA moves multi-KB runs per
partition — a strided gather here is descriptor-bound and ~4x slower. The
gate weights are laid out j-major (all K-slices of one 128-column output
strip adjacent) and streamed strip-by-strip so the first matmul chain waits
only for its own strip, not the whole gate half; hidden states stream
chunk-major on the Scalar engine's DMA queue so their issue overlaps the
weight issues on the Sync queue. Stage 1 computes [f, t] tiles (gate pass
feeding the ScalarE Silu LUT, then up pass fused with the silu output via
scalar_tensor_tensor into bf16 act[i, t]); stage 2 computes out[t, h] with
act as the stationary operand. The down-bias + routing-weight epilogue runs
on the VectorE: out = (psum * w[t]) + w[t]*db[h], with the rank-1 w*db tile
built from a partition-broadcast copy of db.
"""
import sys
if '/opt/trn_rl_repo' not in sys.path:
    sys.path.insert(0, '/opt/trn_rl_repo')
import numpy as np
import ml_dtypes

E, H, I, T = 8, 1024, 1024, 4096
N_CORES = 8
NCH = 8
TC = 512               # tokens per chunk (T == NCH * TC)
KC = H // 128          # contraction chunks (H == I == 1024)
NJ = I // 128          # gate/up row tiles
NTT = TC // 128

_CACHE = {}


def _build():
    import concourse.bacc as bacc
    import concourse.tile as tile
    import concourse.mybir as mybir
    f32 = mybir.dt.float32
    bf16 = mybir.dt.bfloat16
    AF = mybir.ActivationFunctionType
    ALU = mybir.AluOpType

    nc = bacc.Bacc("TRN2", target_bir_lowering=False, debug=False,
                   enable_asserts=False, num_devices=N_CORES)
    # all pre-transposed to SBUF layout on the host (see _make_in_maps)
    hsf = nc.dram_tensor("hsf", [128, NCH * KC * TC], bf16, kind="ExternalInput").ap()
    gupg = nc.dram_tensor("gupg", [128, NJ * KC * 128], bf16, kind="ExternalInput").ap()
    gupu = nc.dram_tensor("gupu", [128, KC * I], bf16, kind="ExternalInput").ap()
    dwTf = nc.dram_tensor("dwTf", [128, KC * H], bf16, kind="ExternalInput").ap()
    gb = nc.dram_tensor("gb", [128, NJ], f32, kind="ExternalInput").ap()
    ub = nc.dram_tensor("ub", [128, NJ], f32, kind="ExternalInput").ap()
    db = nc.dram_tensor("db", [1, H], f32, kind="ExternalInput").ap()
    wt = nc.dram_tensor("wt", [128, T // 128], f32, kind="ExternalInput").ap()
    opart = nc.dram_tensor("opart", [T, H], f32, kind="ExternalOutput").ap()

    SJ = KC * 128        # columns per j-strip of the gate half

    with tile.TileContext(nc) as tc_:
        with tc_.tile_pool(name="wpool", bufs=1) as wpool, \
             tc_.tile_pool(name="hpool", bufs=2) as hpool, \
             tc_.tile_pool(name="apool", bufs=2) as apool, \
             tc_.tile_pool(name="spool", bufs=8) as spool, \
             tc_.tile_pool(name="opool", bufs=3) as opool, \
             tc_.tile_pool(name="bpool", bufs=4) as bpool, \
             tc_.tile_pool(name="ps1", bufs=2, space="PSUM") as ps1, \
             tc_.tile_pool(name="ps2", bufs=4, space="PSUM") as ps2:

            gupg_r = wpool.tile([128, NJ * SJ], bf16)
            gupu_r = wpool.tile([128, KC * I], bf16)
            dwT_r = wpool.tile([128, KC * H], bf16)
            gb_r = wpool.tile([128, NJ], f32)
            ub_r = wpool.tile([128, NJ], f32)
            db_f = wpool.tile([1, H], f32)
            db_bc = wpool.tile([128, H], f32)
            w_r = wpool.tile([128, T // 128], f32)

            # Weight stream on the Sync queue, token stream on the Scalar
            # queue — the ~0.6us per-DMA issue costs overlap. Order matches
            # consumption: j=0 and j=1 gate strips first (the opening matmul
            # chains), biases for the first silu, the remaining gate strips
            # paired, then up / routing / down tensors in use order.
            hs0 = hpool.tile([128, KC * TC], bf16, tag="hs")
            for kc in range(KC):   # per-kc pieces pace the first gate chain
                nc.scalar.dma_start(hs0[:, kc*TC:(kc+1)*TC], hsf[:, kc*TC:(kc+1)*TC])
            hs1 = hpool.tile([128, KC * TC], bf16, tag="hs")
            nc.scalar.dma_start(hs1[:], hsf[:, KC * TC:2 * KC * TC])

            nc.sync.dma_start(gupg_r[:, 0:SJ], gupg[:, 0:SJ])
            nc.sync.dma_start(gupg_r[:, SJ:2 * SJ], gupg[:, SJ:2 * SJ])
            nc.sync.dma_start(gb_r[:], gb[:])
            nc.sync.dma_start(ub_r[:], ub[:])
            for j2 in range(1, NJ // 2):
                nc.sync.dma_start(gupg_r[:, 2*j2*SJ:2*(j2+1)*SJ],
                                  gupg[:, 2*j2*SJ:2*(j2+1)*SJ])
            nc.sync.dma_start(gupu_r[:], gupu[:])
            nc.sync.dma_start(db_f[:], db[:])
            nc.sync.dma_start(w_r[:], wt[:])
            nc.sync.dma_start(dwT_r[:], dwTf[:])
            nc.gpsimd.partition_broadcast(db_bc[:], db_f[:])

            for c in range(NCH):
                if c == 0:
                    hs_r = hs0
                elif c == 1:
                    hs_r = hs1
                else:
                    hs_r = hpool.tile([128, KC * TC], bf16, tag="hs")
                    nc.scalar.dma_start(hs_r[:], hsf[:, c*KC*TC:(c+1)*KC*TC])

                act_r = apool.tile([128, NJ * TC], bf16, tag="act")
                s2s = []
                for j in range(NJ):     # gate pass
                    pg = ps1.tile([128, TC], f32, tag="pg")
                    for kc in range(KC):
                        nc.tensor.matmul(pg[:], gupg_r[:, j*SJ + kc*128 : j*SJ + (kc+1)*128],
                                         hs_r[:, kc*TC:(kc+1)*TC],
                                         start=(kc == 0), stop=(kc == KC - 1))
                    s2 = spool.tile([128, TC], f32, tag="s2")
                    nc.scalar.activation(s2[:], pg[:], AF.Silu,
                                         bias=gb_r[:, j:j+1], scale=1.702)
                    s2s.append(s2)
                for j in range(NJ):     # up pass: act = (up + ub + 1) * silu_out
                    pu = ps1.tile([128, TC], f32, tag="pu")
                    for kc in range(KC):
                        nc.tensor.matmul(pu[:], gupu_r[:, kc*I + j*128 : kc*I + (j+1)*128],
                                         hs_r[:, kc*TC:(kc+1)*TC],
                                         start=(kc == 0), stop=(kc == KC - 1))
                    nc.vector.scalar_tensor_tensor(act_r[:, j*TC:(j+1)*TC], pu[:],
                                                   ub_r[:, j:j+1], s2s[j][:],
                                                   op0=ALU.add, op1=ALU.mult)

                for tt in range(NTT):
                    gt = c * NTT + tt
                    wcol = w_r[:, gt:gt+1]
                    ot = opool.tile([128, H], f32, tag="ot")
                    for hh in range(H // 512):
                        dbw = bpool.tile([128, 512], f32, tag="dbw")
                        nc.vector.tensor_scalar_mul(dbw[:], db_bc[:, hh*512:(hh+1)*512], wcol)
                        p2 = ps2.tile([128, 512], f32, tag="p2")
                        for ic in range(KC):
                            nc.tensor.matmul(p2[:], act_r[:, ic*TC + tt*128 : ic*TC + (tt+1)*128],
                                             dwT_r[:, ic*H + hh*512 : ic*H + (hh+1)*512],
                                             start=(ic == 0), stop=(ic == KC - 1))
                        nc.vector.scalar_tensor_tensor(ot[:, hh*512:(hh+1)*512], p2[:], wcol,
                                                       dbw[:], op0=ALU.mult, op1=ALU.add)
                        nc.sync.dma_start(opart[gt*128:(gt+1)*128, hh*512:(hh+1)*512],
                                          ot[:, hh*512:(hh+1)*512])
    nc.compile()
    return nc


def _get_nc():
    if 'nc' not in _CACHE:
        _CACHE['nc'] = _build()
    return _CACHE['nc']


def _make_in_maps(hidden_states, routing_weights, gate_up_proj, gate_up_proj_bias,
                  down_proj, down_proj_bias):
    bf16 = ml_dtypes.bfloat16
    hs = np.asarray(hidden_states, dtype=np.float32)
    rw = np.asarray(routing_weights, dtype=np.float32)
    gupw = np.asarray(gate_up_proj, dtype=np.float32)
    gupb = np.asarray(gate_up_proj_bias, dtype=np.float32)
    dw = np.asarray(down_proj, dtype=np.float32)
    dbias = np.asarray(down_proj_bias, dtype=np.float32)
    # hsT[kc*128+p, c*TC+t] -> hsf[p, c*(KC*TC) + kc*TC + t]  (chunk-major)
    hsT = hs.T.astype(bf16)
    hsf = np.ascontiguousarray(
        hsT.reshape(KC, 128, NCH, TC).transpose(1, 2, 0, 3).reshape(128, NCH * KC * TC))
    in_maps = []
    for e in range(N_CORES):
        g = gupw[e]
        g_gate = (g[:, 0::2]).astype(bf16)   # [H, I]
        g_up = (g[:, 1::2]).astype(bf16)     # [H, I]
        # g_gate[kc*128+p, j*128+c] -> gupg[p, j*(KC*128) + kc*128 + c]  (j-major)
        gupg = np.ascontiguousarray(
            g_gate.reshape(KC, 128, NJ, 128).transpose(1, 2, 0, 3).reshape(128, NJ * KC * 128))
        # g_up[kc*128+p, col] -> gupu[p, kc*I + col]  (kc-major)
        gupu = np.ascontiguousarray(
            g_up.reshape(KC, 128, I).transpose(1, 0, 2).reshape(128, KC * I))
        # dwT[ic*128+p, h] -> dwTf[p, ic*H + h]; silu's 1.702 scale folded in
        dwT = (dw[e].T / np.float32(1.702)).astype(bf16)
        dwTf = np.ascontiguousarray(
            dwT.reshape(KC, 128, H).transpose(1, 0, 2).reshape(128, KC * H))
        in_maps.append({
            "hsf": hsf,
            "gupg": gupg,
            "gupu": gupu,
            "dwTf": dwTf,
            # silu(1.702*(x + b)) = silu(1.702*x + 1.702*b)
            "gb": np.ascontiguousarray((1.702 * gupb[e, 0::2]).reshape(NJ, 128).T),
            "ub": np.ascontiguousarray((gupb[e, 1::2] + 1.0).reshape(NJ, 128).T),
            "db": np.ascontiguousarray(dbias[e][None, :]),
            "wt": np.ascontiguousarray(rw[:, e].reshape(T // 128, 128).T),
        })
    return in_maps


def _assemble(results):
    out = results[0]["opart"].astype(np.float32, copy=True)
    for r in range(1, N_CORES):
        np.add(out, results[r]["opart"], out=out)
    return out


def kernel(hidden_states, routing_weights, gate_up_proj, gate_up_proj_bias,
           down_proj, down_proj_bias):
    from concourse import bass_utils
    in_maps = _make_in_maps(hidden_states, routing_weights, gate_up_proj,
                            gate_up_proj_bias, down_proj, down_proj_bias)
    nc = _get_nc()
    try:
        res = bass_utils.run_bass_kernel_spmd(nc, in_maps, core_ids=list(range(N_CORES)))
    except Exception:
        # One retry in case a previous process left a core wedged.
        res = bass_utils.run_bass_kernel_spmd(nc, in_maps, core_ids=list(range(N_CORES)))
    return _assemble(res.results)
